# revision 27
# baseline (speedup 1.0000x reference)
"""Trainium2 Bass kernel for grouped-query causal attention (B=2, T=2048, C=1024,
16 q heads / 4 kv heads, RoPE, fused qkv + output projection).

Sharding: 8 cores = (batch b, kv-head h). Each core:
  - projects x -> qT (4 heads), kT, vT with pre-sliced/pre-scaled weights
    (transposed layout: channels on partitions, T on free dim)
  - applies RoPE (pair-swap via permutation matmul on PE + DVE mul/add)
  - causal attention for its 4 query heads (S^T blocks, exp without
    max-subtraction [logits are O(8)], softmax denominators via a ones
    column appended to V, triangular causal mask multiplied on DVE over
    the diagonal 128-col sub-block only; fully-masked columns of diagonal
    tiles are skipped in S/exp/PV)
  - partial output projection y^T = Wf_local^T @ oT  (transposed)
Host sums the 4 per-h partials per batch and transposes back.
"""

import sys

sys.path.insert(0, "/opt/trn_rl_repo")

import ml_dtypes
import numpy as np

import concourse.bacc as bacc
import concourse.mybir as mybir
from concourse import tile
from concourse.bass_utils import run_bass_kernel_spmd

B, T, C = 2, 2048, 1024
G, HKV, HS = 4, 4, 64
OUT_DIM = C + 2 * (C // G)
SCALE = 1.0 / np.sqrt(HS)
MAX_PERIOD = 10000.0

F32 = mybir.dt.float32
F32R = mybir.dt.float32r
BF16 = mybir.dt.bfloat16
AF = mybir.ActivationFunctionType


TCH = T // 512  # 4 chunks of 512 along T
NT = T // 128  # 16 tiles of 128 along T


def build_nc():
    nc = bacc.Bacc(None, target_bir_lowering=False)

    xT_d = nc.dram_tensor("xT", [C, T], F32R, kind="ExternalInput")
    w_d = nc.dram_tensor("w_qkv", [C, 384], F32R, kind="ExternalInput")
    bl_d = nc.dram_tensor("b_loc", [128, 3], F32, kind="ExternalInput")
    cos_d = nc.dram_tensor("cosT", [128, T], BF16, kind="ExternalInput")
    sin_d = nc.dram_tensor("sinT", [128, T], BF16, kind="ExternalInput")
    perm_d = nc.dram_tensor("perm", [128, 128], BF16, kind="ExternalInput")
    eye_d = nc.dram_tensor("eye64", [128, 64], BF16, kind="ExternalInput")
    tri_d = nc.dram_tensor("tri", [128, 2, 128], BF16, kind="ExternalInput")
    wf_d = nc.dram_tensor("wf", [256, 1024], F32R, kind="ExternalInput")
    bf_d = nc.dram_tensor("bf", [128, 8], F32, kind="ExternalInput")
    rsc_d = nc.dram_tensor("rscratch", [16, 512], F32, kind="Internal")
    yT_d = nc.dram_tensor("yT", [C, T], F32, kind="ExternalOutput")

    with tile.TileContext(nc) as tc:
        with (
            tc.tile_pool(name="persist", bufs=1) as pp,
            tc.tile_pool(name="xstream", bufs=16) as spx,
            tc.tile_pool(name="pstream", bufs=19) as spp,
            tc.tile_pool(name="rstream", bufs=3) as spr,
            tc.tile_pool(name="ostream", bufs=3) as spo,
            tc.tile_pool(name="ps_acc", bufs=2, space="PSUM") as psacc,
            tc.tile_pool(name="ps_s", bufs=2, space="PSUM") as pss,
            tc.tile_pool(name="ps_tmp", bufs=2, space="PSUM") as ps,
        ):
            # ---- persistent tiles ----
            w_sb = pp.tile([128, 8, 384], F32R, tag="w", name="w")
            bl_sb = pp.tile([128, 3], F32, tag="bl", name="bl")
            cos_sb = pp.tile([128, T], BF16, tag="cos", name="cos")
            sin_sb = pp.tile([128, T], BF16, tag="sin", name="sin")
            perm_sb = pp.tile([128, 128], BF16, tag="perm", name="perm")
            eye_sb = pp.tile([128, 64], BF16, tag="eye", name="eye")
            tri_sb = pp.tile([128, 2, 128], BF16, tag="tri", name="tri")
            wf_sb = pp.tile([128, 2, 1024], F32R, tag="wf", name="wf")
            bf_sb = pp.tile([128, 8], F32, tag="bf", name="bf")
            qkvT = [pp.tile([128, T], BF16, tag=f"qkvT{m}", name=f"qkvT{m}") for m in range(3)]
            qcat = [pp.tile([64, 2, T], BF16, tag=f"qcat{m}", name=f"qcat{m}") for m in range(2)]
            v_sb = pp.tile([128, NT, 65], BF16, tag="vaug", name="vaug")
            oT_ab = [pp.tile([128, T], F32R, tag=f"oT{i}", name=f"oT{i}") for i in range(2)]

            nc.scalar.dma_start(bl_sb[:], bl_d[:])
            nc.gpsimd.memset(v_sb[:, :, 64:65], 1.0)

            # -- emission helpers ------------------------------------------
            def load_x(tc_i):
                tsl = slice(tc_i * 512, (tc_i + 1) * 512)
                xts = []
                for k in range(8):
                    xt = spx.tile([128, 512], F32R, tag="xt", name="xt")
                    nc.sync.dma_start(xt[:], xT_d[k * 128 : (k + 1) * 128, tsl])
                    xts.append(xt)
                return xts

            def proj_tasks(tc_i, xts):
                """Dense background tasks for chunk tc_i's projection+RoPE+vT."""
                tsl = slice(tc_i * 512, (tc_i + 1) * 512)

                def mk_group(mt):
                    def run():
                        pr = ps.tile([128, 512], F32, tag="tmp", name="tmp")
                        for k in range(8):
                            nc.tensor.matmul(
                                pr[:],
                                w_sb[:, k, mt * 128 : (mt + 1) * 128],
                                xts[k][:],
                                start=(k == 0),
                                stop=(k == 7),
                            )
                        nc.vector.tensor_scalar_add(
                            qkvT[mt][:, tsl], pr[:], bl_sb[:, mt : mt + 1]
                        )
                    return run

                def mk_rope_q(mt):
                    def run():
                        tmp = ps.tile([128, 512], F32, tag="tmp", name="tmp")
                        nc.tensor.matmul(
                            tmp[:], perm_sb[:], qkvT[mt][:, tsl], start=True, stop=True
                        )
                        nc.vector.tensor_mul(
                            qkvT[mt][:, tsl], qkvT[mt][:, tsl], cos_sb[:, tsl]
                        )
                        tmpb = spp.tile([128, 1024], BF16, tag="p", name="p")
                        nc.vector.tensor_mul(tmpb[:, 0:512], tmp[:], sin_sb[:, tsl])
                        nc.vector.tensor_add(
                            qkvT[mt][:, tsl], qkvT[mt][:, tsl], tmpb[:, 0:512]
                        )
                        nc.sync.dma_start(qcat[mt][:, 0, tsl], qkvT[mt][0:64, tsl])
                        nc.sync.dma_start(qcat[mt][:, 1, tsl], qkvT[mt][64:128, tsl])
                    return run

                def mk_vt(i):
                    def run():
                        tt = tc_i * 4 + i
                        vt = ps.tile([128, 512], BF16, tag="tmp", name="tmp")
                        nc.tensor.transpose(
                            vt[:, 0:64],
                            qkvT[2][64:128, tt * 128 : (tt + 1) * 128],
                            eye_sb[64:128, :],
                        )
                        nc.vector.tensor_copy(v_sb[:, tt, 0:64], vt[:, 0:64])
                    return run

                def rope_k():
                    tmp = ps.tile([128, 512], F32, tag="tmp", name="tmp")
                    nc.tensor.matmul(
                        tmp[0:64, :], perm_sb[:, 0:64], qkvT[2][:, tsl],
                        start=True, stop=True,
                    )
                    nc.vector.tensor_mul(
                        qkvT[2][0:64, tsl], qkvT[2][0:64, tsl], cos_sb[0:64, tsl]
                    )
                    tmpb = spp.tile([128, 1024], BF16, tag="p", name="p")
                    nc.vector.tensor_mul(
                        tmpb[0:64, 0:512], tmp[0:64, :], sin_sb[0:64, tsl]
                    )
                    nc.vector.tensor_add(
                        qkvT[2][0:64, tsl], qkvT[2][0:64, tsl], tmpb[0:64, 0:512]
                    )

                return [
                    mk_group(0), mk_group(1), mk_group(2),
                    mk_rope_q(0), mk_rope_q(1),
                    mk_vt(0), mk_vt(1), mk_vt(2), mk_vt(3),
                    rope_k,
                ]

            def final_tasks(tc_i):
                tsl = slice(tc_i * 512, (tc_i + 1) * 512)

                def mk(nt):
                    def run():
                        y_ps = ps.tile([128, 512], F32, tag="tmp", name="tmp")
                        for cc in range(2):
                            nc.tensor.matmul(
                                y_ps[:],
                                wf_sb[:, cc, nt * 128 : (nt + 1) * 128],
                                oT_ab[cc][:, tsl],
                                start=(cc == 0),
                                stop=(cc == 1),
                            )
                        y_sb = spo.tile([128, 512], F32, tag="yout", name="yout")
                        nc.vector.tensor_scalar_add(
                            y_sb[:], y_ps[:], bf_sb[:, nt : nt + 1]
                        )
                        nc.sync.dma_start(yT_d[nt * 128 : (nt + 1) * 128, tsl], y_sb[:])
                    return run

                return [mk(nt) for nt in range(8)]

            def final_tasks_split(tc_i):
                """Epilogue variant: per-nt cc0 (pair-0 heads) can run before
                the last odd-head norm lands; cc1 + drain follows."""
                tsl = slice(tc_i * 512, (tc_i + 1) * 512)
                tiles = {}

                def mk_cc0(nt):
                    def run():
                        y_ps = ps.tile([128, 512], F32, tag="tmp", name="tmp")
                        tiles[nt] = y_ps
                        nc.tensor.matmul(
                            y_ps[:],
                            wf_sb[:, 0, nt * 128 : (nt + 1) * 128],
                            oT_ab[0][:, tsl],
                            start=True,
                            stop=False,
                        )
                    return run

                def mk_cc1(nt):
                    def run():
                        y_ps = tiles[nt]
                        nc.tensor.matmul(
                            y_ps[:],
                            wf_sb[:, 1, nt * 128 : (nt + 1) * 128],
                            oT_ab[1][:, tsl],
                            start=False,
                            stop=True,
                        )
                        y_sb = spo.tile([128, 512], F32, tag="yout", name="yout")
                        nc.vector.tensor_scalar_add(
                            y_sb[:], y_ps[:], bf_sb[:, nt : nt + 1]
                        )
                        nc.sync.dma_start(yT_d[nt * 128 : (nt + 1) * 128, tsl], y_sb[:])
                    return run

                return [mk_cc0(nt) for nt in range(8)], [mk_cc1(nt) for nt in range(8)]

            norm_slot = [0]

            def norm_phase1(o_ac):
                # copy the sums row (psum row 64) to SBUF, then DMA-broadcast
                # to 64 partitions via a DRAM bounce (stride-0 DRAM source).
                # (A custom-DVE recip feeding a DMA is under-synchronized, so
                # the reciprocal happens after the broadcast in phase2.)
                slot = norm_slot[0]
                norm_slot[0] += 1
                rr = spr.tile([65, 512], F32, tag="rrow", name="rr")
                nc.vector.tensor_copy(rr[64:65, :], o_ac[64:65, :])
                nc.sync.dma_start(rsc_d[slot : slot + 1, :], rr[64:65, :])
                s64 = spr.tile([64, 512], F32, tag="s64", name="s64")
                nc.sync.dma_start(
                    s64[:], rsc_d[slot : slot + 1, :].partition_broadcast(64)
                )
                return s64

            def norm_phase2(g, tci_, o_ac, s64):
                # reciprocal + normalize on DVE
                tsl = slice(tci_ * 512, (tci_ + 1) * 512)
                r64 = spr.tile([64, 512], F32, tag="r64", name="r64")
                nc.vector.reciprocal_approx_fast(out=r64[:], in_=s64[:])
                if g % 2 == 0:
                    nc.vector.tensor_mul(
                        oT_ab[g // 2][0:64, tsl], o_ac[0:64, :], r64[:]
                    )
                else:
                    stg = spr.tile([64, 512], F32R, tag="stg", name="stg")
                    nc.vector.tensor_mul(stg[:], o_ac[0:64, :], r64[:])
                    nc.sync.dma_start(oT_ab[g // 2][64:128, tsl], stg[:])

            # -- prologue --------------------------------------------------
            # Parallel descgen: sync queue streams w k0 + x0 + x1 (gating the
            # first proj matmuls); the vector queue loads w k1-7 and the
            # small persistent tensors concurrently.
            nc.sync.dma_start(w_sb[:, 0, :], w_d[0:128, :])
            xts0 = load_x(0)
            nc.scalar.dma_start(
                w_sb[:, 1:8, :],
                w_d[128:1024, :].rearrange("(k p) n -> p k n", p=128),
            )
            nc.scalar.dma_start(cos_sb[:], cos_d[:])
            nc.scalar.dma_start(sin_sb[:], sin_d[:])
            nc.scalar.dma_start(perm_sb[:], perm_d[:])
            nc.scalar.dma_start(eye_sb[:], eye_d[:])
            nc.scalar.dma_start(tri_sb[:], tri_d[:])
            nc.scalar.dma_start(wf_sb[:], wf_d.rearrange("(c p) n -> p c n", p=128))
            nc.scalar.dma_start(bf_sb[:], bf_d[:])
            xts1 = load_x(1)
            p0 = proj_tasks(0, xts0)
            p1 = proj_tasks(1, xts1)
            # interleave: p1's projection groups cover p0's rope/vt latency
            for t in [
                p0[0], p0[1], p0[2], p1[0], p0[3], p1[1], p0[4],
                p0[5], p0[6], p0[7], p0[8], p1[2], p0[9],
                p1[3], p1[4], p1[5], p1[6], p1[7], p1[8], p1[9],
            ]:
                t()

            # -- main loop -------------------------------------------------
            pending_norm = []  # deferred odd-head norm phase2 closures

            for tci in range(TCH):
                tsl = slice(tci * 512, (tci + 1) * 512)
                nblk = 4 * tci + 4

                bg = []
                if tci + 2 < TCH:
                    xts = load_x(tci + 2)
                    bg += proj_tasks(tci + 2, xts)
                if tci == 2:
                    bg += final_tasks(0) + final_tasks(1)
                elif tci == 3:
                    bg += final_tasks(2)
                bg_done = 0
                bg_total = len(bg)
                slots = G * nblk
                slot = 0

                for pair in range(2):
                    qc = qcat[pair]
                    o_acs = [
                        psacc.tile([128, 512], F32, tag="oacc", name="oacc")
                        for _ in range(2)
                    ]
                    DEPTH = 3
                    pq = []       # (j, p_view, off) waiting for h0 PV
                    plist = []    # all (j, p_view, off) for h1's dense tail

                    def emit_pv(jj, h01, p_tile, off, o_acs=o_acs, nblk=nblk):
                        nc.tensor.matmul(
                            o_acs[h01][0:65, off:512],
                            v_sb[:, jj, 0:65],
                            p_tile[:, h01, off:512],
                            start=(jj == 0),
                            stop=(jj == nblk - 1),
                        )

                    for j in range(nblk):
                        m = j - 4 * tci  # >= 0 on diagonal tiles
                        off = 128 * m if m >= 0 else 0
                        s_ps = pss.tile([128, 1024], F32, tag="s", name="s")
                        for h01 in range(2):
                            nc.tensor.matmul(
                                s_ps[:, h01 * 512 + off : (h01 + 1) * 512],
                                qkvT[2][0:64, j * 128 : (j + 1) * 128],
                                qc[:, h01, tci * 512 + off : (tci + 1) * 512],
                                start=True,
                                stop=True,
                            )
                        p_sb = spp.tile([128, 1024], BF16, tag="p", name="p")
                        sv = s_ps[:].rearrange("q (h t) -> q h t", h=2)
                        pp_view = p_sb[:].rearrange("q (h t) -> q h t", h=2)
                        nc.scalar.activation(
                            pp_view[:, :, off:512], sv[:, :, off:512], AF.Exp
                        )
                        if m >= 0:
                            nc.vector.tensor_mul(
                                pp_view[:, :, off : off + 128],
                                pp_view[:, :, off : off + 128],
                                tri_sb[:],
                            )
                        pq.append((j, pp_view, off))
                        plist.append((j, pp_view, off))
                        if len(pq) > DEPTH:
                            jj, pv, o_ = pq.pop(0)
                            emit_pv(jj, 0, pv, o_)
                        if pending_norm and j == 1:
                            pending_norm.pop(0)()
                        slot += 2
                        due = bg_total * min(slot, slots) // slots
                        while bg_done < due:
                            bg[bg_done]()
                            bg_done += 1

                    for jj, pv, o_ in pq:
                        emit_pv(jj, 0, pv, o_)
                    # even-head norm: sums-row copy now, broadcast+normalize
                    # a few tail PVs later (keeps the PE from waiting on it)
                    rr0 = norm_phase1(o_acs[0])
                    pos2 = nblk - 1 if nblk <= 4 else 5
                    # head 1: dense back-to-back PV run
                    for ti, (jj, pv, o_) in enumerate(plist):
                        emit_pv(jj, 1, pv, o_)
                        if ti == pos2:
                            norm_phase2(pair * 2, tci, o_acs[0], rr0)
                    # odd head norm: phase1 now, phase2 in next pair's j-loop
                    rr1 = norm_phase1(o_acs[1])
                    g_odd = pair * 2 + 1
                    o_ac1 = o_acs[1]
                    tci_c = tci
                    pending_norm.append(
                        lambda g=g_odd, t_=tci_c, o=o_ac1, r=rr1: norm_phase2(
                            g, t_, o, r
                        )
                    )

                while bg_done < bg_total:
                    bg[bg_done]()
                    bg_done += 1

            # -- epilogue: overlap last chunk's final proj with the last
            # odd-head norm (cc0 only needs pair-0 heads)
            cc0s, cc1s = final_tasks_split(TCH - 1)
            cc0s[0]()
            cc0s[1]()
            for fn in pending_norm:
                fn()
            for n in range(8):
                cc1s[n]()
                if n + 2 < 8:
                    cc0s[n + 2]()

    nc.compile()
    return nc


def host_shard(inputs):
    """Build the 8 per-core input maps from full inputs."""
    x = np.ascontiguousarray(np.asarray(inputs["input"], dtype=np.float32))
    W = np.asarray(inputs["W_attn"], dtype=np.float32)
    bb = np.asarray(inputs["b_attn"], dtype=np.float32)
    Wf = np.asarray(inputs["W_final"], dtype=np.float32)
    bf = np.asarray(inputs["b_final"], dtype=np.float32)

    half = HS // 2
    inv_freq = MAX_PERIOD ** (-np.arange(half, dtype=np.float32) / half)
    ang = np.arange(T, dtype=np.float32)[:, None] * inv_freq  # (T, 32)
    sin_t = np.sin(ang).astype(np.float32)
    cos_t = np.cos(ang).astype(np.float32)
    cosT = np.repeat(cos_t.T, 2, axis=0)  # (64, T): row d -> cos(t*f[d//2])
    sgn = np.where(np.arange(HS) % 2 == 0, -1.0, 1.0).astype(np.float32)
    sinT = np.repeat(sin_t.T, 2, axis=0) * sgn[:, None]
    cos128 = np.ascontiguousarray(np.concatenate([cosT, cosT], axis=0))
    sin128 = np.ascontiguousarray(np.concatenate([sinT, sinT], axis=0))

    perm = np.zeros((128, 128), np.float32)
    idx = np.arange(128)
    perm[idx ^ 1, idx] = 1.0
    eye64 = np.zeros((128, 64), np.float32)
    eye64[64:128, :] = np.eye(64, dtype=np.float32)
    # triangular causal mask for the diagonal 128-col sub-block:
    # keep p[kv_row, t] iff t >= kv_row  (t local to the 128-block)
    tr = (np.arange(128)[None, :] >= np.arange(128)[:, None]).astype(np.float32)
    tri = np.ascontiguousarray(np.stack([tr, tr], axis=1))  # (128, 2, 128)

    in_maps = []
    for cid in range(8):
        b, h = cid // 4, cid % 4
        qcols = np.concatenate(
            [np.arange(g * 256 + h * 64, g * 256 + h * 64 + 64) for g in range(G)]
        )
        kcols = np.arange(1024 + h * 64, 1024 + h * 64 + 64)
        vcols = np.arange(1280 + h * 64, 1280 + h * 64 + 64)
        cols = np.concatenate([qcols, kcols, vcols])
        w_loc = W[:, cols].copy()
        b_loc = bb[cols].copy()
        w_loc[:, :256] *= SCALE
        b_loc[:256] *= SCALE
        b_loc_m = np.ascontiguousarray(b_loc.reshape(3, 128).T)  # (128, 3)

        rows = np.concatenate(
            [np.arange(g * 256 + h * 64, g * 256 + h * 64 + 64) for g in range(G)]
        )
        wf_loc = np.ascontiguousarray(Wf[rows, :])  # (256, 1024)
        bf_m = (
            np.ascontiguousarray(bf.reshape(8, 128).T)
            if h == 0
            else np.zeros((128, 8), np.float32)
        )

        in_maps.append(
            {
                "xT": np.ascontiguousarray(x[b].T),
                "w_qkv": w_loc,
                "b_loc": b_loc_m,
                "cosT": cos128.astype(ml_dtypes.bfloat16),
                "sinT": sin128.astype(ml_dtypes.bfloat16),
                "perm": perm.astype(ml_dtypes.bfloat16),
                "eye64": eye64.astype(ml_dtypes.bfloat16),
                "tri": tri.astype(ml_dtypes.bfloat16),
                "wf": wf_loc,
                "bf": bf_m,
            }
        )
    return in_maps


def host_unshard(results):
    """Sum the 4 per-h partial yT per batch, transpose back to (B, T, C)."""
    out = np.empty((B, T, C), np.float32)
    for b in range(B):
        acc = results[b * 4]["yT"].astype(np.float32)
        for h in range(1, 4):
            acc = acc + results[b * 4 + h]["yT"]
        out[b] = acc.T
    return out


_NC_CACHE = None


def _get_nc():
    global _NC_CACHE
    if _NC_CACHE is None:
        _NC_CACHE = build_nc()
    return _NC_CACHE


def kernel(**inputs):
    nc = _get_nc()
    in_maps = host_shard(inputs)
    res = run_bass_kernel_spmd(nc, in_maps, core_ids=list(range(8)))
    return host_unshard(res.results)
